# revision 26
# baseline (speedup 1.0000x reference)
"""Trainium2 Bass kernel for masked-softmax attention scoring.

Reference computation (B=128, T=512, K=1024, Q=1024):
    mids  = einsum("kq,bq->bk", W, query)
    s     = tanh(einsum("btk,bk->bt", key, mids) + bias)
    attn  = softmax-like: exp(s - max) * mask / sum(exp(s - max) * mask)

The max-subtraction cancels exactly in the ratio (tanh is bounded), so the
device computes  attn = exp(tanh(.)) * mask / sum_t(exp(tanh(.)) * mask).

Sharding: data-parallel over B across 8 NeuronCores (16 batches/core).

v12 design ("masked row packing", evolved from v8; v12 = final):
  * The kernel is HBM-streaming bound: v8's trace shows the DMA stream at a
    sustained ~350 GB/s with zero mid-stream gaps, so the only wins left are
    (a) fewer bytes and (b) a shorter post-stream tail.
  * (a) Row packing: scores are needed only where mask==1 (~80% of rows).
    The mask is host-visible, so the host packs only the kept key rows.
    Packing must be uniform across the 8 SPMD cores (one program), so a
    shared template gives each batch a fixed row-region sized by the max
    kept count over cores (<=2 wasted blocks/core vs. tight packing).
    Key traffic drops 16.78 -> ~14.2 MB/core (~ -7.5 us of stream).
  * Rows pack into 128-row blocks; each block spans at most 2 consecutive
    batches (every batch keeps >=~380 rows >> 128).  Per block j the PE
    does, per k-chunk, one 128x128 fp16 stationary load (the key rows) and
    a 2-column moving pass with mids[:, bj:bj+2] -> scores[t, (j, c)]
    accumulated over the 8 k-chunks in one PSUM bank per group.
  * Two batch-groups per core (15 + 1): group 1 (1 batch, ~4 blocks)
    keeps the final serial epilogue small.  ((14,2) with more tail cover
    for chain 0 measured WORSE, 65.8us vs 62.9us; both chains largely
    land in the tail either way and the ScalarE overlap wins out.)
  * Epilogue per group (all shapes [128, NB2], NB2 = 2*NB <= 128):
    tanh (ScalarE, from PSUM, +bias) -> exp fp16 -> sel-mask multiply
    (DVE; sel zeroes wrong-batch columns and pad rows) -> column sums via
    ONE matmul with em as the 128xNB2 stationary and a ones column ->
    denominators via ONE matmul with the (NB2 x nbat) 0/1 column->batch
    map as stationary -> reciprocal fp16 -> map^T * rden (DVE broadcast)
    -> rank-1 broadcast matmul to [128, NB2] -> final scale (DVE) -> DMA.
    Wrong-column / pad entries are dropped by the host-side scatter.
  * DMA rings: sync carries W half 0 + even key pieces, scalar W half 1 +
    odd pieces (enqueue order == consumption order per ring); the
    otherwise-idle GpSimd software ring carries every small load (qt,
    bias, sel, maps), so no key-piece descriptor generation ever sits
    behind a data-dependent wait.  Group-1 key is stored kc-minor in two
    kc-halves, each ONE dma with ~4KB/partition runs: 1KB runs measured
    ~20GB/s slower aggregate (v11), and with this layout the stream
    sustains ~359 GB/s.  Output DMAs go at the END of the sync/scalar
    programs (queues drained by then; doorbell ~0.7-0.9us on any ring).
  * Chain PE matmuls are spread across group-1 stream positions (hooks)
    so PE never head-of-line blocks on the DVE/ScalarE chain ops.
  * fp16 wire format everywhere big (key, W^T): raw scores have std ~59
    and tanh saturates hard; measured rel-l2 ~1.4e-3 vs the 2e-2 budget
    (fp8 key fails: ~4.5e-2 measured in v8 experiments).
"""

import sys

if "/opt/trn_rl_repo" not in sys.path:
    sys.path.insert(0, "/opt/trn_rl_repo")

from contextlib import ExitStack

import numpy as np

# ---- problem constants (hardcoded per spec) ----
B, T, K, Q = 128, 512, 1024, 1024
NCORES = 8
BS = B // NCORES          # 16 batches per core
P = 128                   # SBUF partitions
KC = K // P               # 8 contraction chunks for the scores matmuls
QC = Q // P               # 8 contraction chunks for the mids matmuls
MB = BS + 1               # mids batch columns (1 zero pad for block pairs)

GROUPS = [(0, 15), (15, 16)]   # local batch ranges per group (big, tiny-tail)
K0_BUFS = 6               # group-0 key piece pool depth
K1_BUFS = 8               # group-1 key piece pool depth (all pieces resident)
HOOKS = (3, 5, 6)         # group-1 stream positions for chain-0 PE matmuls

_STATE: dict = {}


def _plan_from_mask(mask):
    """Template packing plan shared by all 8 cores (SPMD: one program)."""
    kept = mask.sum(axis=1).astype(np.int64).reshape(NCORES, BS)
    tmpl_len = kept.max(axis=0)          # rows reserved per local batch
    assert tmpl_len.min() >= P, "packing assumes every batch keeps >=128 rows"
    plan = []
    for (b0, b1) in GROUPS:
        lens = tmpl_len[b0:b1]
        starts = np.concatenate([[0], np.cumsum(lens)])
        NB = int(np.ceil(starts[-1] / P))
        assert 2 * NB <= P, "em stationary must fit 128 columns"
        bj = [
            min(int(np.searchsorted(starts, j * P, side="right")) - 1,
                len(starts) - 2)
            for j in range(NB)
        ]
        plan.append((b0, b1, starts, NB, tuple(bj)))
    return plan


def _build_nc(plan):
    import concourse.tile as tile
    from concourse import bacc, mybir

    f32 = mybir.dt.float32
    f16 = mybir.dt.float16
    nc = bacc.Bacc()

    NBs = [pl[3] for pl in plan]
    nbats = [pl[1] - pl[0] for pl in plan]

    # group 0: kc-major pieces (runs of NB0*128*2 ~ 12.8KB/partition).
    # group 1: kc-minor within two kc-halves, so each half is ONE dma with
    # 4*NB1*128*2 ~ 4KB/partition runs (1KB runs measured ~20GB/s slower).
    kp0_e = nc.declare_dram_parameter(
        "keyp0", [KC, P, NBs[0] * P], f16, isOutput=False
    )
    kp1_e = nc.declare_dram_parameter(
        "keyp1", [2, P, KC // 2, NBs[1] * P], f16, isOutput=False
    )
    # wt[h, qp, qh, kc, kl] = W[kc*128 + kl, (h*4 + qh)*128 + qp]
    wt_e = nc.declare_dram_parameter("wt", [2, P, QC // 2, KC, P], f16, isOutput=False)
    qt_e = nc.declare_dram_parameter("qt", [P, QC, MB], f16, isOutput=False)
    bias_e = nc.declare_dram_parameter("biasb", [P, 1], f32, isOutput=False)
    sel_es = [
        nc.declare_dram_parameter(f"sel{g}", [P, NBs[g] * 2], f16, isOutput=False)
        for g in range(2)
    ]
    map_es = [
        nc.declare_dram_parameter(f"map{g}", [NBs[g] * 2, nbats[g]], f16, isOutput=False)
        for g in range(2)
    ]
    mapT_es = [
        nc.declare_dram_parameter(f"mapT{g}", [nbats[g], NBs[g] * 2], f16, isOutput=False)
        for g in range(2)
    ]
    out_es = [
        nc.declare_dram_parameter(f"out{g}", [P, NBs[g] * 2], f16, isOutput=True)
        for g in range(2)
    ]

    with tile.TileContext(nc) as tc, ExitStack() as ctx:
        const = ctx.enter_context(tc.tile_pool(name="const", bufs=1))
        kpool0 = ctx.enter_context(tc.tile_pool(name="key0", bufs=K0_BUFS))
        kpool1 = ctx.enter_context(tc.tile_pool(name="key1", bufs=K1_BUFS))
        psum = ctx.enter_context(tc.tile_pool(name="psum", bufs=1, space="PSUM"))

        rings = [nc.sync, nc.scalar]

        # W halves first, one per key ring (their transfers head both queues)
        wt_sbs = [
            const.tile([P, QC // 2, KC, P], f16, tag=f"wt{h}", name=f"wt{h}")
            for h in range(2)
        ]
        nc.sync.dma_start(out=wt_sbs[0][:], in_=wt_e[0])
        nc.scalar.dma_start(out=wt_sbs[1][:], in_=wt_e[1])

        # every small load rides the otherwise-idle GpSimd ring (own queue,
        # drains in parallel with W; keeps key rings' descriptor gen clean)
        qt_sb = const.tile([P, QC, MB], f16)
        bias_sb = const.tile([P, 1], f32)
        nc.gpsimd.dma_start(out=qt_sb[:], in_=qt_e[:])
        nc.gpsimd.dma_start(out=bias_sb[:], in_=bias_e[:])
        sel_sbs, map_sbs, mapT_sbs = [], [], []
        for g in range(2):
            s = const.tile([P, NBs[g] * 2], f16, tag=f"sel{g}", name=f"sel{g}")
            m = const.tile([NBs[g] * 2, nbats[g]], f16, tag=f"map{g}", name=f"map{g}")
            mt = const.tile([nbats[g], NBs[g] * 2], f16, tag=f"mapT{g}", name=f"mapT{g}")
            nc.gpsimd.dma_start(out=s[:], in_=sel_es[g][:])
            nc.gpsimd.dma_start(out=m[:], in_=map_es[g][:])
            nc.gpsimd.dma_start(out=mt[:], in_=mapT_es[g][:])
            sel_sbs.append(s); map_sbs.append(m); mapT_sbs.append(mt)

        ones_col = const.tile([P, 1], f16)
        nc.vector.memset(ones_col[:], 1.0)
        ones_bat = const.tile([BS, P], f16)
        nc.vector.memset(ones_bat[:], 1.0)

        # all key piece dma_starts upfront; pool rotation paces the
        # sequencers, enqueue order per ring == consumption order.  (v10
        # tried splitting group-1 pieces across both rings to balance the
        # queue drain; the smaller per-partition runs cost ~10 GB/s of
        # aggregate DMA efficiency and it measured net-negative.  All 16
        # DMA engines serve whichever queue has work, so per-ring byte
        # imbalance does not waste aggregate bandwidth.)
        pieces0 = [None] * KC
        for kc in range(KC):
            t = kpool0.tile([P, NBs[0] * P], f16, tag="k0", name=f"k0_{kc}")
            rings[kc % 2].dma_start(out=t[:], in_=kp0_e[kc])
            pieces0[kc] = t
        pieces1 = [None, None]
        for h in range(2):
            t = kpool1.tile(
                [P, KC // 2, NBs[1] * P], f16, tag="k1", name=f"k1_{h}"
            )
            rings[h % 2].dma_start(out=t[:], in_=kp1_e[h])
            pieces1[h] = t

        # ---- mids^T[k, (kc, b)] = sum_q W[k, q] query[b, q]  (b has pad col)
        mids_ps = psum.tile([P, KC, MB], f32)
        for qi, (h, qh) in enumerate(
            [(0, 0), (0, 1), (0, 2), (0, 3), (1, 0), (1, 1), (1, 2), (1, 3)]
        ):
            for kc in range(KC):
                nc.tensor.matmul(
                    mids_ps[:, kc, :],
                    lhsT=wt_sbs[h][:, qh, kc, :],
                    rhs=qt_sb[:, h * (QC // 2) + qh, :],
                    start=(qi == 0 and kc == 0),
                    stop=(qi == QC - 1),
                )
        mids_sb = const.tile([P, KC, MB], f16)
        nc.vector.tensor_copy(mids_sb[:], mids_ps[:])

        # ---- scores[t, (j, c)] += key-block^T @ mids[:, bj:bj+2] ----
        sc_ts = [
            psum.tile([P, NBs[g] * 2], f32, tag=f"sc{g}", name=f"sc{g}")
            for g in range(2)
        ]

        def emit_stream(g, hook=None):
            b0, b1, starts, NB, bj = plan[g]
            for kc in range(KC):
                if hook is not None:
                    hook(kc)
                for j in range(NB):
                    if g == 0:
                        lhsT = pieces0[kc][:, j * P : (j + 1) * P]
                    else:
                        lhsT = pieces1[kc // (KC // 2)][
                            :, kc % (KC // 2), j * P : (j + 1) * P
                        ]
                    mb0 = b0 + bj[j]
                    nc.tensor.matmul(
                        sc_ts[g][:, 2 * j : 2 * j + 2],
                        lhsT=lhsT,
                        rhs=mids_sb[:, kc, mb0 : mb0 + 2],
                        start=(kc == 0 and j == 0),
                        stop=(kc == KC - 1),
                    )

        # epilogue chain, split so its PE matmuls can be spread via hooks
        def chain_pre(g):
            NB2 = plan[g][3] * 2
            th = const.tile([P, NB2], f32, tag=f"th{g}", name=f"th{g}")
            nc.scalar.activation(
                out=th[:],
                in_=sc_ts[g][:],
                func=mybir.ActivationFunctionType.Tanh,
                bias=bias_sb[:],
                scale=1.0,
            )
            ex = const.tile([P, NB2], f16, tag=f"ex{g}", name=f"ex{g}")
            nc.scalar.activation(
                out=ex[:], in_=th[:], func=mybir.ActivationFunctionType.Exp
            )
            em = const.tile([P, NB2], f16, tag=f"em{g}", name=f"em{g}")
            nc.vector.tensor_tensor(em[:], ex[:], sel_sbs[g][:], mybir.AluOpType.mult)
            return em

        def chain_colsums(g, em):
            NB2 = plan[g][3] * 2
            cs_ps = psum.tile([P, 1], f32, tag="cs", name=f"cs{g}")
            nc.tensor.matmul(
                cs_ps[:NB2, :], lhsT=em[:], rhs=ones_col[:], start=True, stop=True
            )
            cs_sb = const.tile([P, 1], f16, tag=f"csb{g}", name=f"csb{g}")
            nc.vector.tensor_copy(cs_sb[:NB2, :], cs_ps[:NB2, :])
            return cs_sb

        def chain_denom(g, cs_sb):
            NB2 = plan[g][3] * 2
            nbat = nbats[g]
            dn_ps = psum.tile([BS, 1], f32, tag="dn", name=f"dn{g}")
            nc.tensor.matmul(
                dn_ps[:nbat, :],
                lhsT=map_sbs[g][:],
                rhs=cs_sb[:NB2, :],
                start=True,
                stop=True,
            )
            rden = const.tile([BS, 1], f16, tag=f"rd{g}", name=f"rd{g}")
            with nc.allow_low_precision(reason="1/denom fp16: rel 5e-4 << 2e-2"):
                nc.vector.reciprocal(out=rden[:nbat, :], in_=dn_ps[:nbat, :])
            rdmap = const.tile([nbat, NB2], f16, tag=f"rm{g}", name=f"rm{g}")
            nc.vector.tensor_tensor(
                rdmap[:],
                mapT_sbs[g][:],
                rden[:nbat, :].broadcast_to((nbat, NB2)),
                mybir.AluOpType.mult,
            )
            return rdmap

        def chain_scale(g, em, rdmap):
            NB2 = plan[g][3] * 2
            nbat = nbats[g]
            rb_ps = psum.tile([P, NB2], f32, tag=f"rb{g}", name=f"rb{g}")
            nc.tensor.matmul(
                rb_ps[:], lhsT=ones_bat[:nbat, :], rhs=rdmap[:], start=True, stop=True
            )
            attn = const.tile([P, NB2], f16, tag=f"at{g}", name=f"at{g}")
            with nc.allow_low_precision(reason="attn fp16 out: rel 5e-4 << 2e-2"):
                nc.vector.tensor_tensor(attn[:], em[:], rb_ps[:], mybir.AluOpType.mult)
            # outputs ride the by-now-drained key hardware rings (the gpsimd
            # software queue has ~1.3us doorbell latency, measured v9)
            rings[g].dma_start(out=out_es[g][:], in_=attn[:])

        emit_stream(0)
        em0 = chain_pre(0)
        h = {}

        def hook(kc):
            if kc == HOOKS[0]:
                h["cs"] = chain_colsums(0, em0)
            elif kc == HOOKS[1]:
                h["rm"] = chain_denom(0, h["cs"])
            elif kc == HOOKS[2]:
                chain_scale(0, em0, h["rm"])

        emit_stream(1, hook)
        em1 = chain_pre(1)
        cs1 = chain_colsums(1, em1)
        rm1 = chain_denom(1, cs1)
        chain_scale(1, em1, rm1)

    nc.compile()
    return nc


def _get_nc(plan):
    key = tuple((pl[0], pl[1], pl[3], pl[4]) for pl in plan)
    if _STATE.get("key") != key:
        _STATE["nc"] = _build_nc(plan)
        _STATE["key"] = key
    return _STATE["nc"]


def _make_in_maps(query, key, mask, W, bias):
    query = np.asarray(query, dtype=np.float32)
    key = np.asarray(key, dtype=np.float32)
    mask = np.asarray(mask, dtype=np.float32)
    W = np.asarray(W, dtype=np.float32)
    bias = np.asarray(bias, dtype=np.float32).reshape(-1)

    plan = _plan_from_mask(mask)
    _STATE["plan"] = plan

    # wt[h, qp, qh, kc, kl] = W[kc*128 + kl, (h*4 + qh)*128 + qp]
    WT = np.ascontiguousarray(
        W.T.astype(np.float16).reshape(2, QC // 2, P, KC, P).transpose(0, 2, 1, 3, 4)
    )
    biasb = np.ascontiguousarray(
        np.broadcast_to(bias[:1][None, :], (P, 1)).astype(np.float32)
    )
    key16 = key.astype(np.float16)

    # shared per-group map matrices (template-determined, same for all cores)
    maps, mapTs = [], []
    for (b0, b1, starts, NB, bj) in plan:
        nbat = b1 - b0
        mp = np.zeros((NB * 2, nbat), np.float16)
        for j in range(NB):
            for cc in range(2):
                bb = bj[j] + cc
                if bb < nbat:
                    mp[2 * j + cc, bb] = 1.0
        maps.append(mp)
        mapTs.append(np.ascontiguousarray(mp.T))

    in_maps = []
    scatter = []   # per core, per group: (rows_pos, lb, t) index arrays
    for c in range(NCORES):
        m = {"wt": WT, "biasb": biasb}
        qh = query[c * BS : (c + 1) * BS].T.astype(np.float16)   # [Q, BS]
        qtp = np.zeros((Q, MB), np.float16)
        qtp[:, :BS] = qh
        m["qt"] = np.ascontiguousarray(qtp.reshape(QC, P, MB).transpose(1, 0, 2))
        sc_g = []
        for g, (b0, b1, starts, NB, bj) in enumerate(plan):
            R = NB * P
            buf = np.zeros((R, K), np.float16)
            sel = np.zeros((P, NB, 2), np.float16)
            r_list, lb_list, t_list = [], [], []
            for lb in range(b1 - b0):
                gb = c * BS + b0 + lb
                ts = np.nonzero(mask[gb])[0]
                r0 = int(starts[lb])
                rr = r0 + np.arange(len(ts))
                buf[rr] = key16[gb, ts]
                jj, tp = rr // P, rr % P
                cc = lb - np.asarray(bj)[jj]
                sel[tp, jj, cc] = 1.0
                r_list.append(rr); lb_list.append(np.full(len(ts), lb)); t_list.append(ts)
            kt_kc = np.ascontiguousarray(buf.T).reshape(KC, P, R)
            if g == 0:
                m["keyp0"] = kt_kc
            else:
                # [2 halves, P, KC//2, R]: kc-minor inside each half
                m["keyp1"] = np.ascontiguousarray(
                    kt_kc.reshape(2, KC // 2, P, R).transpose(0, 2, 1, 3)
                )
            m[f"sel{g}"] = np.ascontiguousarray(sel.reshape(P, NB * 2))
            m[f"map{g}"] = maps[g]
            m[f"mapT{g}"] = mapTs[g]
            sc_g.append(
                (np.concatenate(r_list), np.concatenate(lb_list), np.concatenate(t_list))
            )
        in_maps.append(m)
        scatter.append(sc_g)
    _STATE["scatter"] = scatter
    return in_maps


def _run(in_maps, **kwargs):
    from concourse.bass_utils import run_bass_kernel_spmd

    return run_bass_kernel_spmd(
        _get_nc(_STATE["plan"]), in_maps, core_ids=list(range(NCORES)), **kwargs
    )


def _gather(results):
    plan = _STATE["plan"]
    scatter = _STATE["scatter"]
    attn = np.zeros((B, T), dtype=np.float32)
    for c, r in enumerate(results):
        for g, (b0, b1, starts, NB, bj) in enumerate(plan):
            out = np.asarray(r[f"out{g}"])          # [P, NB*2]
            rr, lb, ts = scatter[c][g]
            jj, tp = rr // P, rr % P
            cc = lb - np.asarray(bj)[jj]
            attn[c * BS + b0 + lb, ts] = out[tp, 2 * jj + cc]
    return attn


def kernel(query, key, mask, W, bias):
    in_maps = _make_in_maps(query, key, mask, W, bias)
    res = _run(in_maps)
    return _gather(res.results)
